# revision 34
# baseline (speedup 1.0000x reference)
"""Chunked cross-attention (retrieval KNN) Trainium2 Bass kernel.

Problem shapes: x [2048, 1024], neighbours [32, 2, 512, 1024],
Wq/Wk/Wv/Wo [1024, 1024]; 64-token chunks, 2 neighbours x 512 tokens,
16 heads x 64 head-dim; softmax over the QUERY axis (source quirk).

Distribution: data-parallel over the 31 "main" chunks across 8 cores
(cores 0-6: 4 chunks, core 7: 3 chunks + a duplicated dummy), weights
replicated. The degenerate last-token chunk (single query => softmax
over one element => uniform weights) reduces to
(0.5 * sum_kr neighbours[31]) @ Wv.T @ Wo.T and is computed on-device
in a small epilogue block (only core 7's copy is used).

On-chip dataflow per chunk (all layouts pre-transposed on host so the
contraction dim lands on SBUF partitions). The two projection GEMMs --
the arithmetic bulk -- run as fp8e4 DoubleRow (double-pumped) matmuls:
ctx is host-quantized at x16, Wk/Wv at x512 (keeps everything in e4m3
normals, |x| < 240), and the 1/8192 dequant folds into wqT / woT:
  kkT[l, kr] = sum_d Wk8T[d, l] * ctx8T[d, kr]   (fp8 DoubleRow, N=512)
  vv[kr, l]  = sum_d ctx8T[d, kr]^T * Wv8T[d, l] (fp8 DoubleRow, N=512)
  qT[l, q]   = sum_d WqT[d, l] * attT[d, q]      (bf16, N=256)
  scores[r, q] per (k,h) = kkT^T q               (bf16 matmuls, N=64)
  softmax over q (free dim): exp -> per-(h,r) row-sum -> reciprocal
  o[q, l] += w'[r, q]^T vv[r, l] over (k, r)     (bf16, N=64)
  z[q, d] = sum_l oT[l, q] * WoT[l, d]           (f32r, N=512; 0.5 & mean
                                                  over k folded into WoT)
"""

import numpy as np
from contextlib import ExitStack

import concourse.bass as bass
import concourse.tile as tile
from concourse import bacc, mybir, masks
from concourse import bass_utils

F32 = mybir.dt.float32
F32R = mybir.dt.float32r
BF16 = mybir.dt.bfloat16
F8 = mybir.dt.float8e4
EXP = mybir.ActivationFunctionType.Exp
AX_X = mybir.AxisListType.X
MULT = mybir.AluOpType.mult
DR = mybir.MatmulPerfMode.DoubleRow

N, M, K, R, D = 2048, 64, 2, 512, 1024
H, L = 16, 1024
LCH = N // M          # 32 chunks
DH = L // H           # 64
SCALE = 1.0 / (D ** 0.5)
NCORES = 8
NSLOT = 4             # chunk slots per core
DT = 8                # 128-row tiles along the d (contraction) dim
DP = 4                # fp8 DoubleRow d-tile pairs
CS = 16.0             # host fp8 scale on ctx
WS = 512.0            # host fp8 scale on Wk/Wv

# chunk assignment: cores 0-6 -> 4 chunks each (0..27), core 7 -> 28,29,30 + dup(30)
CORE_CHUNKS = [[4 * c + i for i in range(4)] for c in range(7)] + [[28, 29, 30, 30]]

_nc_cache = None


def _build_program():
    nc = bacc.Bacc("TRN2", target_bir_lowering=False, debug=False, enable_asserts=False)
    ctx8T = nc.dram_tensor("ctx8T", [NSLOT, DP, 128, 2048], F8, kind="ExternalInput").ap()
    attT = nc.dram_tensor("attT", [128, DT, NSLOT * M], BF16, kind="ExternalInput").ap()
    wqT = nc.dram_tensor("wqT", [128, DT, 1024], BF16, kind="ExternalInput").ap()
    wk8T = nc.dram_tensor("wk8T", [DP, 128, 2048], F8, kind="ExternalInput").ap()
    wv8T = nc.dram_tensor("wv8T", [DP, 128, 2048], F8, kind="ExternalInput").ap()
    woT = nc.dram_tensor("woT", [DT, 128, 1024], F32R, kind="ExternalInput").ap()
    c31T = nc.dram_tensor("ctx31T", [DT, 128, 1024], F32, kind="ExternalInput").ap()
    wvloT = nc.dram_tensor("wvloT", [DT, 128, 1024], F8, kind="ExternalInput").ap()
    zout = nc.dram_tensor("z", [NSLOT, M, 1024], F32, kind="ExternalOutput").ap()
    z31out = nc.dram_tensor("z31", [1, 1024], F32, kind="ExternalOutput").ap()

    def pair(ap):
        # [128, 2048] fp8 tile -> [128, 2 (d-pair), 1024] DoubleRow view
        return ap.rearrange("p (two f) -> p two f", two=2)

    with tile.TileContext(nc) as tc, ExitStack() as ctx:
        def pool(name, bufs, space=bass.MemorySpace.SBUF):
            return ctx.enter_context(tc.tile_pool(name=name, bufs=bufs, space=space))

        wkp = pool("wk", 4)
        wvp = pool("wv", 4)
        attp = pool("att", 1)
        wqp = pool("wq", 1)
        woqp = pool("woq", 8)
        ctxp = pool("ctx", 8)
        c31p = pool("c31", 5)
        qtp = pool("qt", 16)
        kkp = pool("kk", 9)
        vvp = pool("vv", 9)
        wrp = pool("wr", 2)
        wsp = pool("ws", 16)           # w' tiles; also hosts the att tiles
        dnp = pool("dn", 4)
        rcp = pool("rc", 4)
        otp = pool("ot", 2)
        zsbp = pool("zsb", 2)
        s31p = pool("s31", 1)
        vstp = pool("vst", 1)
        wvlp = pool("wvl", 8)
        mmps = pool("mmps", 2, space=bass.MemorySpace.PSUM)
        scps = pool("scps", 4, space=bass.MemorySpace.PSUM)
        ops = pool("ops", 2, space=bass.MemorySpace.PSUM)

        _dma_engines = (nc.sync, nc.gpsimd)

        def dma(i, out_ap, in_ap):
            # stripe DGE rings so startup loads stream in parallel
            _dma_engines[i % 2].dma_start(out_ap, in_ap)

        # DMA issue order = need order: wk+ctx0 (the kk matmuls start the
        # PE stream), then att+wq (qT runs under kk(0)), then wv, wo, wvlo.
        # Each dma_start's descriptors spread over all 16 DGE rings, so
        # full-tile transfers (2KB+ lines, one DIRECT2D) beat split ones.
        wk_sb, wv_sb = [], []
        for dp in range(DP):
            t = wkp.tile([128, 2048], F8, tag="wk", name=f"wk{dp}")
            dma(dp, t[:], wk8T[dp])
            wk_sb.append(t)

        def load_ctx(s):
            # mid-stream loads issue on sync only: gpsimd's queue carries the
            # softmax reduces, which must never delay DMA descriptor issue
            tiles = []
            for dp in range(DP):
                t = ctxp.tile([128, 2048], F8, tag="ctx", name=f"ctx{s}_{dp}")
                if s == 0:
                    dma(dp, t[:], ctx8T[s, dp])
                else:
                    nc.sync.dma_start(t[:], ctx8T[s, dp])
                tiles.append(t)
            return tiles

        # kkT[l, kr] via fp8 DoubleRow: each matmul contracts a d-tile PAIR.
        def emit_kk(s, ctx_sb):
            kk_sb = []
            for lt in range(DT):
                kt = kkp.tile([128, 1024], BF16, tag="kk", name=f"kk{s}_{lt}")
                for half in range(2):
                    ps = mmps.tile([128, 512], F32, tag="mm", name=f"kkps{s}_{lt}_{half}")
                    for dp in range(DP):
                        nc.tensor.matmul(ps[:],
                                         pair(wk_sb[dp][:])[:, :, lt * 128:(lt + 1) * 128],
                                         pair(ctx_sb[dp][:])[:, :, half * 512:(half + 1) * 512],
                                         start=(dp == 0), stop=(dp == DP - 1),
                                         perf_mode=DR)
                    nc.scalar.copy(kt[:, half * 512:(half + 1) * 512], ps[:])
                kk_sb.append(kt)
            return kk_sb

        ctx0_sb = load_ctx(0)
        # merged SBUF tiles, but per-d DMA slices: small descriptors spread
        # over both issue queues, so they don't HOL-block the ctx0 stream
        att_sb = attp.tile([128, DT, NSLOT * M], BF16, tag="att", name="att")
        for d in range(DT):
            dma(d, att_sb[:, d, :], attT[:, d, :])
        wq_sb = wqp.tile([128, DT, 1024], BF16, tag="wq", name="wq")
        for d in range(DT):
            dma(d, wq_sb[:, d, :], wqT[:, d, :])
        kk0_sb = emit_kk(0, ctx0_sb)

        # qT[l, (slot, q)] for all 4 slots at once. Two zero-padded
        # variants per l-tile (even heads live in partitions 0-63, odd in
        # 64-127; the other half is zeroed) so every score matmul contracts
        # over the full 128 partitions at base partition 0 -- consecutive
        # PE matmuls with differing base partitions hard-fault the exec unit.
        qt_e, qt_o = [], []
        for lt in range(DT):
            ps = mmps.tile([128, NSLOT * M], F32, tag="mm", name=f"qtps{lt}")
            for d in range(DT):
                nc.tensor.matmul(ps[:],
                                 wq_sb[:, d, lt * 128:(lt + 1) * 128],
                                 att_sb[:, d, :],
                                 start=(d == 0), stop=(d == DT - 1))
            qe = qtp.tile([128, NSLOT * M], BF16, tag="qt", name=f"qte{lt}")
            nc.gpsimd.memset(qe[64:128, :], 0.0)
            nc.scalar.copy(qe[0:64, :], ps[0:64, :])
            qt_e.append(qe)
            qo = qtp.tile([128, NSLOT * M], BF16, tag="qt", name=f"qto{lt}")
            nc.gpsimd.memset(qo[0:64, :], 0.0)
            nc.scalar.copy(qo[64:128, :], ps[64:128, :])
            qt_o.append(qo)

        for dp in range(DP):
            t = wvp.tile([128, 2048], F8, tag="wv", name=f"wv{dp}")
            dma(2 * dp, t[:, 0:1024], wv8T[dp][:, 0:1024])
            dma(2 * dp + 1, t[:, 1024:2048], wv8T[dp][:, 1024:2048])
            wv_sb.append(t)
        wo_sb = []
        for d in range(DT):
            t = woqp.tile([128, 1024], F32R, tag="woq", name=f"wo{d}")
            dma(d, t[:], woT[d])
            wo_sb.append(t)
        wvlo_sb = []
        for d in range(DT):
            t = wvlp.tile([128, 1024], F8, tag="wvl", name=f"wvlo{d}")
            dma(d, t[:], wvloT[d])
            wvlo_sb.append(t)

        def emit_vv(s, ctx_sb):
            vv_sb = []
            for rt2 in range(8):
                vt = vvp.tile([128, 1024], BF16, tag="vv", name=f"vv{s}_{rt2}")
                for half in range(2):
                    ps = mmps.tile([128, 512], F32, tag="mm", name=f"vvps{s}_{rt2}_{half}")
                    for dp in range(DP):
                        nc.tensor.matmul(ps[:],
                                         pair(ctx_sb[dp][:])[:, :, rt2 * 128:(rt2 + 1) * 128],
                                         pair(wv_sb[dp][:])[:, :, half * 512:(half + 1) * 512],
                                         start=(dp == 0), stop=(dp == DP - 1),
                                         perf_mode=DR)
                    nc.scalar.copy(vt[:, half * 512:(half + 1) * 512], ps[:])
                vv_sb.append(vt)
            return vv_sb

        def emit_scores(s, kk_sb):
            # scores -> exp -> row-sum over q (free dim) -> normalize
            w_sb = {}
            gi = 0
            for k in range(2):
                for rt in range(4):
                    for hh in range(2):
                        sps = scps.tile([128, 512], F32, tag="sc", name=f"sc{s}_{k}{rt}{hh}")
                        for hi in range(8):
                            h = hh * 8 + hi
                            lt = h // 2
                            qt = qt_e[lt] if h % 2 == 0 else qt_o[lt]
                            nc.tensor.matmul(
                                sps[:, hi * 64:(hi + 1) * 64],
                                kk_sb[lt][:, k * 512 + rt * 128:k * 512 + (rt + 1) * 128],
                                qt[:, s * M:(s + 1) * M],
                                start=True, stop=True)
                        wr = wrp.tile([128, 512], BF16, tag="wr", name=f"wr{s}_{k}{rt}{hh}")
                        nc.scalar.activation(wr[:], sps[:], EXP)
                        dn = dnp.tile([128, 8], F32, tag="dn", name=f"dn{s}_{k}{rt}{hh}")
                        nc.vector.reduce_sum(
                            dn[:], wr[:].rearrange("p (h q) -> p h q", h=8), axis=AX_X)
                        rc = rcp.tile([128, 8], F32, tag="rc", name=f"rc{s}_{k}{rt}{hh}")
                        nc.vector.reciprocal(rc[:], dn[:])
                        ws = wsp.tile([128, 512], BF16, tag="ws", name=f"ws{s}_{k}{rt}{hh}")
                        # normalize on the idle Q7s: keeps the vector softmax
                        # chain (exp-sum + recip) under the kk PE cover, so
                        # emit_o never waits on the last w' tile
                        eng = nc.gpsimd
                        eng.tensor_tensor(
                            ws[:].rearrange("p (h q) -> p h q", h=8),
                            wr[:].rearrange("p (h q) -> p h q", h=8),
                            rc[:].unsqueeze(2).broadcast_to([128, 8, 64]),
                            op=MULT)
                        gi += 1
                        w_sb[(k, rt, hh)] = ws
            return w_sb

        def emit_o(s, w_sb, vv_sb):
            # oT[l, q] accumulated directly transposed (lhsT=vv, rhs=w'):
            # psum [128, 512] = 1 bank, rows = l within l-tile (even head in
            # partitions 0-63, odd in 64-127), cols = (lt, q). Groups are
            # sequential start..stop so zero-region groups never interleave.
            o_ps = ops.tile([128, 512], F32, tag="o", name=f"ops{s}")
            for lt in range(DT):
                for par in range(2):
                    h = 2 * lt + par
                    hh, hi = h // 8, h % 8
                    poff = 64 * par
                    n = 0
                    for k in range(2):
                        for rt in range(4):
                            nc.tensor.matmul(
                                o_ps[poff:poff + 64, lt * 64:(lt + 1) * 64],
                                vv_sb[k * 4 + rt][:, h * 64:(h + 1) * 64],
                                w_sb[(k, rt, hh)][:, hi * 64:(hi + 1) * 64],
                                start=(n == 0), stop=(n == 7))
                            n += 1
            ot = otp.tile([128, 512], F32R, tag="ot", name=f"ot{s}")
            nc.scalar.copy(ot[:], o_ps[:])
            return ot

        def emit_z(s, ot):
            # per-half DMA so half 0 streams out under half 1's matmuls --
            # trims the exposed tail after the last slot
            z_sb = zsbp.tile([M, 1024], F32, tag="zsb", name=f"zsb{s}")
            for half in range(2):
                ps = mmps.tile([M, 512], F32, tag="mm", name=f"zps{s}_{half}")
                for lt in range(DT):
                    nc.tensor.matmul(ps[:],
                                     ot[:, lt * 64:(lt + 1) * 64],
                                     wo_sb[lt][:, half * 512:(half + 1) * 512],
                                     start=(lt == 0), stop=(lt == DT - 1))
                nc.vector.tensor_copy(z_sb[:, half * 512:(half + 1) * 512], ps[:])
                nc.sync.dma_start(zout[s][:, half * 512:(half + 1) * 512],
                                  z_sb[:, half * 512:(half + 1) * 512])

        # ---- last-token chunk: z31 = (sum_kr ctx31) @ WvT @ WoT ----
        # split into load/compute halves, traced ~100us apart so the serial
        # reduction chain never stalls the PE stream
        z31_state = {}

        def z31_load(part):
            # split across two slots so the vector reduces never queue ahead
            # of a full softmax chain
            if part == 0:
                c31_sb = []
                for d in range(DT // 2):
                    t = c31p.tile([128, 1024], F32, tag="c31", name=f"c31_{d}")
                    nc.sync.dma_start(t[:], c31T[d])
                    c31_sb.append(t)
                s31f = s31p.tile([128, DT], F32, tag="s31f")
                for d in range(DT // 2):
                    nc.vector.reduce_sum(s31f[:, d:d + 1], c31_sb[d][:], axis=AX_X)
                z31_state["s31f"] = s31f
                return
            s31f = z31_state["s31f"]
            c31_sb = []
            for d in range(DT // 2, DT):
                t = c31p.tile([128, 1024], F32, tag="c31", name=f"c31_{d}")
                nc.sync.dma_start(t[:], c31T[d])
                c31_sb.append(t)
            for d in range(DT // 2, DT):
                nc.vector.reduce_sum(s31f[:, d:d + 1],
                                     c31_sb[d - DT // 2][:], axis=AX_X)
            # compensated bf16: s31 ~ hi + lo recovers near-fp32 input
            # precision for this row (its operands are ~30x larger than the
            # main path's). Columns duplicated: small matmuls need a moving
            # free dim >= 2.
            s31h = s31p.tile([128, 2 * DT], BF16, tag="s31h")
            nc.scalar.copy(s31h[:].rearrange("p (d two) -> p d two", two=2),
                           s31f[:].unsqueeze(2).broadcast_to([128, DT, 2]))
            s31r = s31p.tile([128, DT], F32, tag="s31res")
            nc.vector.tensor_sub(s31r[:], s31f[:], s31h[:].rearrange(
                "p (d two) -> p d two", two=2)[:, :, 0:1].squeeze(2))
            s31l = s31p.tile([128, 2 * DT], BF16, tag="s31l")
            nc.scalar.copy(s31l[:].rearrange("p (d two) -> p d two", two=2),
                           s31r[:].unsqueeze(2).broadcast_to([128, DT, 2]))
            z31_state["s31h"] = s31h
            z31_state["s31l"] = s31l

        def z31_mm():
            # vst = s31 @ Wv8 (+ s31h @ wvlo residual correction for the fp8
            # quantization of Wv); mixed fp8-lhsT x bf16-rhs matmuls.
            s31h, s31l = z31_state["s31h"], z31_state["s31l"]

            def wv8_slice(d, lt):
                return pair(wv_sb[d // 2][:])[:, d % 2, lt * 128:(lt + 1) * 128]

            vst = vstp.tile([128, DT], F32R, tag="vst")
            for lt in range(DT):
                ps = mmps.tile([128, 2], F32, tag="mm", name=f"vstps{lt}")
                n = 0
                for wsel, st in (("v8", s31h), ("v8", s31l), ("lo", s31h)):
                    for d in range(DT):
                        lhs = wv8_slice(d, lt) if wsel == "v8" else \
                            wvlo_sb[d][:, lt * 128:(lt + 1) * 128]
                        nc.tensor.matmul(ps[:], lhs, st[:, 2 * d:2 * d + 2],
                                         start=(n == 0), stop=(n == 3 * DT - 1))
                        n += 1
                nc.scalar.copy(vst[:, lt:lt + 1], ps[:, 0:1])
            z31_sb = zsbp.tile([M, 1024], F32, tag="zsb", name="z31sb")
            for half in range(2):
                ps = mmps.tile([1, 512], F32, tag="mm", name=f"z31ps{half}")
                for lt in range(DT):
                    nc.tensor.matmul(ps[:],
                                     vst[:, lt:lt + 1],
                                     wo_sb[lt][:, half * 512:(half + 1) * 512],
                                     start=(lt == 0), stop=(lt == DT - 1))
                nc.vector.tensor_copy(z31_sb[0:1, half * 512:(half + 1) * 512], ps[:])
            nc.sync.dma_start(z31out[:], z31_sb[0:1, :])

        # ---- software-pipelined emission: next chunk's projections fill
        # the softmax / output-transpose dependency shadows ----
        ctx_sb = {0: ctx0_sb}
        kk = {0: kk0_sb}
        vv = {0: emit_vv(0, ctx0_sb)}
        for s in range(NSLOT):
            w_sb = emit_scores(s, kk.pop(s))
            if s in (0, 1):
                z31_load(s)
            if s + 1 < NSLOT:
                ctx_sb[s + 1] = load_ctx(s + 1)
                kk[s + 1] = emit_kk(s + 1, ctx_sb[s + 1])
            if s == 2:
                z31_mm()
            ot = emit_o(s, w_sb, vv[s])
            if s + 1 < NSLOT:
                vv[s + 1] = emit_vv(s + 1, ctx_sb.pop(s + 1))
            emit_z(s, ot)
            vv.pop(s)

    nc.compile()
    return nc


def _get_program():
    global _nc_cache
    if _nc_cache is None:
        _nc_cache = _build_program()
    return _nc_cache


def _pack8(a):
    # [1024 d, 1024 f] fp8 -> [DP, 128, 2048] with d-tile pairs interleaved
    # in the free dim: partition p of pair dp holds [d=2dp*128+p | d=(2dp+1)*128+p]
    return np.ascontiguousarray(
        a.reshape(DP, 2, 128, 1024).transpose(0, 2, 1, 3).reshape(DP, 128, 2048))


def _prep_inputs(x, neighbours, Wq, Wk, Wv, Wo):
    import ml_dtypes
    bf16 = ml_dtypes.bfloat16
    f8 = ml_dtypes.float8_e4m3
    x = np.ascontiguousarray(np.asarray(x, dtype=np.float32))
    neighbours = np.ascontiguousarray(np.asarray(neighbours, dtype=np.float32))
    inv = 1.0 / (CS * WS)
    wqT = np.ascontiguousarray(
        (np.asarray(Wq, np.float32).T * (SCALE * inv))
        .astype(bf16).reshape(DT, 128, 1024).transpose(1, 0, 2))
    wk8 = np.clip(np.asarray(Wk, np.float32).T * WS, -240, 240).astype(f8)
    wk8T = _pack8(wk8)
    wv8 = np.clip(np.asarray(Wv, np.float32).T * WS, -240, 240).astype(f8)
    wv8T = _pack8(wv8)
    wvloT = np.clip((np.asarray(Wv, np.float32).T * WS) - wv8.astype(np.float32),
                    -240, 240).astype(f8).reshape(DT, 128, 1024)
    woT = np.ascontiguousarray(
        np.asarray(Wo, np.float32).T * (0.5 * inv)).reshape(DT, 128, 1024)

    zeros31 = np.zeros((DT, 128, 1024), np.float32)
    c31 = np.ascontiguousarray(
        neighbours[31].reshape(1024, 1024).T * CS).reshape(DT, 128, 1024)

    in_maps = []
    for c in range(NCORES):
        chunks = CORE_CHUNKS[c]
        att = np.concatenate(
            [x[M - 1 + M * u: M - 1 + M * (u + 1)] for u in chunks], axis=0)  # [256, 1024]
        attT = np.ascontiguousarray(
            att.T.astype(bf16).reshape(DT, 128, NSLOT * M).transpose(1, 0, 2))
        ctx8T = np.stack(
            [_pack8(np.clip(neighbours[u].reshape(1024, 1024).T * CS,
                            -240, 240).astype(f8))
             for u in chunks])
        in_maps.append({
            "ctx8T": ctx8T,
            "attT": attT,
            "wqT": wqT, "wk8T": wk8T, "wv8T": wv8T, "woT": woT, "wvloT": wvloT,
            "ctx31T": c31 if c == NCORES - 1 else zeros31,
        })
    return x, in_maps


def _assemble(x, results):
    out = np.empty((N, D), np.float32)
    out[:M - 1] = x[:M - 1]
    done = set()
    for c in range(NCORES):
        for si, u in enumerate(CORE_CHUNKS[c]):
            if u in done:
                continue
            done.add(u)
            out[M - 1 + M * u: M - 1 + M * (u + 1)] = results[c]["z"][si]
    out[N - 1] = results[NCORES - 1]["z31"][0]
    return out


def _run(x, in_maps, trace=False):
    nc = _get_program()
    res = bass_utils.run_bass_kernel_spmd(nc, in_maps, core_ids=list(range(NCORES)),
                                          trace=trace)
    return res


def kernel(x, neighbours, Wq, Wk, Wv, Wo):
    x, in_maps = _prep_inputs(x, neighbours, Wq, Wk, Wv, Wo)
    res = _run(x, in_maps, trace=False)
    return _assemble(x, res.results)


def kernel_timed(x, neighbours, Wq, Wk, Wv, Wo):
    """Same as kernel() but also returns the profiled HW execution time (ns)."""
    x, in_maps = _prep_inputs(x, neighbours, Wq, Wk, Wv, Wo)
    res = _run(x, in_maps, trace=True)
    return _assemble(x, res.results), res.exec_time_ns
